# revision 15
# baseline (speedup 1.0000x reference)
"""DepthScaleShiftInvLoss kernel for one TRN2 chip (8 NeuronCores).

Full inputs: pred/gt f32 [32,512,512], mask bool [32,512,512].
Output: dense masked loss f32 [32,512,512] (zeros off-mask).

Sharding: pure data parallel — batch dim split 4 samples/core across 8 cores,
no cross-core communication.

History: baseline 76us (f32 I/O, all DMA on one queue, all stats passes on
ACT) -> v2 25.7us (bf16 I/O halves traffic; DMA split over both HWDGE
queues; fused stage-C bias) -> v4 (this file): the v2 profile was
ACT-engine-bound (4 big activation passes/sample), so the abs-sum passes
move to DVE as scalar_tensor_tensor((x - shift) * mf) + tensor_reduce(abs)
— which also deletes the off-mask correction terms — stage-C's two
elementwise ops fuse into one scalar_tensor_tensor, and the final mask
multiply runs on the otherwise-idle GpSimd (Pool) engine. ACT keeps two
big passes per sample (mask u8->bf16 cast with count accumulation, and the
fused |a*x + b| output pass).

Per-core pipeline (SBUF layout per sample is [128 partitions x 2048],
partition p holding image rows [4p, 4p+4); samples are independent chains,
emitted staggered so one sample's stats barriers overlap other samples'
bulk work):

  stage A   ACT: mf = bf16(mask_u8) (Copy, accum -> count)
            DVE: pm = pred*mf, gm = gt*mf (bf16)
            PE:  masked sums of pm/gm via 16 accumulating matmuls each
                 (data stationary, ones moving) -> PSUM [128,1] partials
  barrier1  PE: ones[128,128] @ partials -> totals replicated across
            partitions; cnt=max(c,1), invc=1/cnt, sp, sg (tiny DVE ops).
  stage B   DVE: cen = (pm - sp)*mf   (scalar_tensor_tensor; off-mask 0)
            DVE: tensor_reduce abs-sum -> per-partition partials
  barrier2  PE matmul folds partials; scp=max(sum*invc,EPS), a=1/scp,
            r=scp/scg, bias2=a*(sp-r*sg) (tiny DVE ops).
  stage C   DVE: w = r*gm - pm        (scalar_tensor_tensor)
            ACT: o1 = |a*w + bias2|   (== a*|pm - sp - r*(gt - sg)| on-mask)
            Pool: out = o1*mf -> bf16, DMA out
"""

import numpy as np
import ml_dtypes

import concourse.bass as bass
import concourse.bacc as bacc
import concourse.tile as tile
from concourse import mybir
from concourse.bass_utils import run_bass_kernel_spmd

B, H, W = 32, 512, 512
N_CORES = 8
B_LOC = B // N_CORES          # samples per core
P = 128                       # SBUF partitions
FD = (H // P) * W             # free elements per sample per partition
N_ELEM = float(H * W)         # elements per sample
EPS = 1e-6

f32 = mybir.dt.float32
bf16 = mybir.dt.bfloat16
u8 = mybir.dt.uint8

ALU = mybir.AluOpType
ACTF = mybir.ActivationFunctionType
AXL = mybir.AxisListType

# engine placement knobs (tuned empirically)
ABS_P = "dve"        # "dve": STT+reduce   "act": activation Abs+accum
ABS_G = "dve"
OUTF = "dve"         # final mask multiply on "pool" or "dve"


class _PerSample:
    __slots__ = ("mf", "pm", "gm", "pc", "pp", "pg", "p2",
                 "cnt", "invc", "sp", "sg", "a_p", "r_t", "bias2",
                 "corr_p", "corr_g")


def build_body(nc):
    pred = nc.dram_tensor("pred", [B_LOC, H, W], bf16, kind="ExternalInput").ap()
    gt = nc.dram_tensor("gt", [B_LOC, H, W], bf16, kind="ExternalInput").ap()
    mask = nc.dram_tensor("mask", [B_LOC, H, W], u8, kind="ExternalInput").ap()
    out = nc.dram_tensor("out", [B_LOC, H, W], bf16, kind="ExternalOutput").ap()

    # [a, (p r), w] -> [p, a, (r w)]: per (partition, sample) 2048 contiguous
    # elements in DRAM.
    pr = pred.rearrange("a (p r) w -> p a (r w)", p=P)
    gr = gt.rearrange("a (p r) w -> p a (r w)", p=P)
    mr = mask.rearrange("a (p r) w -> p a (r w)", p=P)
    outr = out.rearrange("a (p r) w -> p a (r w)", p=P)

    LAST = B_LOC - 1

    with tile.TileContext(nc) as tc:
        with (
            tc.tile_pool(name="io", bufs=3) as io,
            tc.tile_pool(name="keep", bufs=B_LOC) as keep,
            tc.tile_pool(name="tmp", bufs=2) as tmp,
            tc.tile_pool(name="small", bufs=B_LOC) as small,
            tc.tile_pool(name="ps", bufs=2, space="PSUM") as ps,
            tc.tile_pool(name="const", bufs=1) as const,
        ):
            ones = const.tile([P, P], f32)
            nc.vector.memset(ones, 1.0)
            ones_b = const.tile([P, 1], bf16)
            nc.vector.memset(ones_b, 1.0)

            def pe_sum(big, psum_acc):
                # total-sum helper: 16 accumulating matmuls with the data as
                # the stationary operand; psum_acc[m] = sum_p,k big[p, 128k+m]
                for k in range(0, FD, P):
                    nc.tensor.matmul(psum_acc, big[:, k:k + P], ones_b,
                                     start=(k == 0), stop=(k == FD - P))

            S = [_PerSample() for _ in range(B_LOC)]

            def stage_mask(s):
                st = S[s]
                m_in = io.tile([P, FD], u8, tag="m_in", bufs=B_LOC,
                               name=f"m_in{s}")
                nc.sync.dma_start(out=m_in, in_=mr[:, s, :])
                st.pc = small.tile([P, 1], f32, tag="pc", name=f"pc{s}")
                st.mf = keep.tile([P, FD], bf16, tag="mf", name=f"mf{s}")
                nc.scalar.activation(out=st.mf, in_=m_in, func=ACTF.Copy,
                                     accum_out=st.pc)

            def stage_a(s):
                st = S[s]
                p_in = io.tile([P, FD], bf16, tag="p_in", name=f"p_in{s}")
                nc.sync.dma_start(out=p_in, in_=pr[:, s, :])
                g_in = io.tile([P, FD], bf16, tag="g_in", name=f"g_in{s}")
                nc.scalar.dma_start(out=g_in, in_=gr[:, s, :])

                st.pp = small.tile([P, 1], f32, tag="pp", name=f"pp{s}")
                st.pg = small.tile([P, 1], f32, tag="pg", name=f"pg{s}")
                st.pm = keep.tile([P, FD], bf16, tag="pm", name=f"pm{s}")
                nc.vector.tensor_tensor(st.pm, p_in, st.mf, ALU.mult)
                st.gm = keep.tile([P, FD], bf16, tag="gm", name=f"gm{s}")
                nc.vector.tensor_tensor(st.gm, g_in, st.mf, ALU.mult)
                # masked sums on the TensorEngine (per-partition partials)
                psum_pp = ps.tile([P, 1], f32, tag="psum_pp", name=f"pspp{s}")
                pe_sum(st.pm, psum_pp)
                nc.scalar.copy(out=st.pp, in_=psum_pp)
                psum_pg = ps.tile([P, 1], f32, tag="psum_pg", name=f"pspg{s}")
                pe_sum(st.gm, psum_pg)
                nc.scalar.copy(out=st.pg, in_=psum_pg)

            def barrier1(s):
                st = S[s]
                late = s == LAST
                psum1 = ps.tile([P, 3], f32, tag="psum1", name=f"ps1_{s}")
                nc.tensor.matmul(psum1[:, 0:1], ones, st.pc, start=True, stop=True)
                nc.tensor.matmul(psum1[:, 1:2], ones, st.pp, start=True, stop=True)
                nc.tensor.matmul(psum1[:, 2:3], ones, st.pg, start=True, stop=True)
                stats1 = small.tile([P, 3], f32, tag="stats1", name=f"st1_{s}")
                if late:
                    nc.vector.tensor_copy(stats1, psum1)
                else:
                    nc.scalar.copy(out=stats1, in_=psum1)

                st.cnt = small.tile([P, 1], f32, tag="cnt", name=f"cnt{s}")
                st.invc = small.tile([P, 1], f32, tag="invc", name=f"invc{s}")
                st.sp = small.tile([P, 1], f32, tag="sp", name=f"sp{s}")
                st.sg = small.tile([P, 1], f32, tag="sg", name=f"sg{s}")
                eng = nc.vector
                eng.tensor_scalar(st.cnt, stats1[:, 0:1], 1.0, None, ALU.max)
                nc.vector.reciprocal(st.invc, st.cnt)
                eng.tensor_tensor(st.sp, stats1[:, 1:2], st.invc, ALU.mult)
                eng.tensor_tensor(st.sg, stats1[:, 2:3], st.invc, ALU.mult)

            def _abs_dve(st, s, src, shift, dst):
                cen = tmp.tile([P, FD], bf16, tag="cen", name=f"cen{s}_{id(src)%97}")
                nc.vector.scalar_tensor_tensor(cen, src, shift, st.mf,
                                               ALU.subtract, ALU.mult)
                nc.vector.tensor_reduce(dst, cen, AXL.XYZW, ALU.add,
                                        apply_absolute_value=True)

            def _abs_act(st, s, src, shift, dst, corr_attr):
                # activation path needs the off-mask correction: accum
                # counts (N-cnt) copies of |shift|
                scr = tmp.tile([P, FD], bf16, tag="scr", name=f"scr{s}_{id(src)%97}")
                nc.scalar.activation(out=scr, in_=src, func=ACTF.Abs,
                                     bias=shift, scale=-1.0, accum_out=dst)
                ash = small.tile([P, 1], f32, tag=f"ash{corr_attr}",
                                 name=f"ash{corr_attr}{s}")
                nc.scalar.activation(out=ash, in_=shift, func=ACTF.Abs)
                offc = small.tile([P, 1], f32, tag=f"offc{corr_attr}",
                                  name=f"offc{corr_attr}{s}")
                nc.vector.tensor_scalar(offc, st.cnt, -1.0, N_ELEM,
                                        ALU.mult, ALU.add)
                corr = small.tile([P, 1], f32, tag=f"corr{corr_attr}",
                                  name=f"corr{corr_attr}{s}")
                nc.vector.tensor_tensor(corr, offc, ash, ALU.mult)
                setattr(st, corr_attr, corr)

            def stage_b(s):
                st = S[s]
                st.p2 = small.tile([P, 2], f32, tag="p2", name=f"p2_{s}")
                if ABS_P == "dve":
                    _abs_dve(st, s, st.pm, st.sp, st.p2[:, 0:1])
                    st.corr_p = None
                else:
                    _abs_act(st, s, st.pm, st.sp, st.p2[:, 0:1], "corr_p")
                if ABS_G == "dve":
                    _abs_dve(st, s, st.gm, st.sg, st.p2[:, 1:2])
                    st.corr_g = None
                else:
                    _abs_act(st, s, st.gm, st.sg, st.p2[:, 1:2], "corr_g")

            def barrier2(s):
                st = S[s]
                late = s == LAST
                psum2 = ps.tile([P, 2], f32, tag="psum2", name=f"ps2_{s}")
                nc.tensor.matmul(psum2, ones, st.p2, start=True, stop=True)
                stats2 = small.tile([P, 2], f32, tag="stats2", name=f"st2_{s}")
                if late:
                    nc.vector.tensor_copy(stats2, psum2)
                else:
                    nc.scalar.copy(out=stats2, in_=psum2)

                eng = nc.vector
                sums = [stats2[:, 0:1], stats2[:, 1:2]]
                for i, corr in enumerate([st.corr_p, st.corr_g]):
                    if corr is not None:
                        num = small.tile([P, 1], f32, tag=f"num{i}",
                                         name=f"num{i}_{s}")
                        eng.tensor_tensor(num, sums[i], corr, ALU.subtract)
                        sums[i] = num
                scp = small.tile([P, 1], f32, tag="scp", name=f"scp{s}")
                eng.tensor_scalar(scp, sums[0], st.invc, EPS, ALU.mult, ALU.max)
                scg = small.tile([P, 1], f32, tag="scg", name=f"scg{s}")
                eng.tensor_scalar(scg, sums[1], st.invc, EPS, ALU.mult, ALU.max)
                st.a_p = small.tile([P, 1], f32, tag="a_p", name=f"ap{s}")
                nc.vector.reciprocal(st.a_p, scp)
                i_g = small.tile([P, 1], f32, tag="i_g", name=f"ig{s}")
                nc.vector.reciprocal(i_g, scg)
                st.r_t = small.tile([P, 1], f32, tag="r_t", name=f"rt{s}")
                eng.tensor_tensor(st.r_t, scp, i_g, ALU.mult)
                rsg = small.tile([P, 1], f32, tag="rsg", name=f"rsg{s}")
                eng.tensor_tensor(rsg, st.r_t, st.sg, ALU.mult)
                # bias2 = a*(sp - r*sg): o1 = |a*(r*gm - pm) + bias2|
                qd = small.tile([P, 1], f32, tag="qd", name=f"qd{s}")
                eng.tensor_tensor(qd, st.sp, rsg, ALU.subtract)
                st.bias2 = small.tile([P, 1], f32, tag="bias2", name=f"b2{s}")
                eng.tensor_tensor(st.bias2, qd, st.a_p, ALU.mult)

            def stage_c(s):
                st = S[s]
                w = tmp.tile([P, FD], bf16, tag="w", name=f"w{s}")
                nc.vector.scalar_tensor_tensor(w, st.gm, st.r_t, st.pm,
                                               ALU.mult, ALU.subtract)
                o1 = tmp.tile([P, FD], bf16, tag="o1", name=f"o1{s}")
                nc.scalar.activation(out=o1, in_=w, func=ACTF.Abs,
                                     scale=st.a_p, bias=st.bias2)
                outf = tmp.tile([P, FD], bf16, tag="outf", name=f"outf{s}")
                if OUTF == "pool":
                    nc.gpsimd.tensor_tensor(outf, o1, st.mf, ALU.mult)
                else:
                    nc.vector.tensor_tensor(outf, o1, st.mf, ALU.mult)
                if s % 2 == 0:
                    nc.sync.dma_start(out=outr[:, s, :], in_=outf)
                else:
                    nc.scalar.dma_start(out=outr[:, s, :], in_=outf)

            # Emission order == scheduling priority. Masks/casts first, the
            # pred/gt stream + TTRs next, then each sample's stats/B staged
            # ahead of earlier samples' C.
            for s in range(B_LOC):
                stage_mask(s)
            stage_a(0)
            stage_a(1)
            barrier1(0)
            stage_b(0)
            stage_a(2)
            barrier1(1)
            stage_b(1)
            barrier2(0)
            stage_a(3)
            barrier1(2)
            stage_b(2)
            barrier2(1)
            stage_c(0)
            barrier1(3)
            stage_b(3)
            barrier2(2)
            stage_c(1)
            barrier2(3)
            stage_c(2)
            stage_c(3)
    return nc


_CACHED = None


def _get_nc():
    global _CACHED
    if _CACHED is None:
        nc = bacc.Bacc("TRN2", target_bir_lowering=False, debug=False)
        build_body(nc)
        nc.compile()
        _CACHED = nc
    return _CACHED


def kernel(pred: np.ndarray, gt: np.ndarray, mask: np.ndarray) -> np.ndarray:
    pred = np.ascontiguousarray(np.asarray(pred).astype(ml_dtypes.bfloat16))
    gt = np.ascontiguousarray(np.asarray(gt).astype(ml_dtypes.bfloat16))
    mask = np.asarray(mask)
    mask_u8 = np.ascontiguousarray(
        mask.view(np.uint8) if mask.dtype == np.bool_ else mask.astype(np.uint8)
    )

    nc = _get_nc()
    in_maps = []
    for c in range(N_CORES):
        lo, hi = c * B_LOC, (c + 1) * B_LOC
        in_maps.append(
            {"pred": pred[lo:hi], "gt": gt[lo:hi], "mask": mask_u8[lo:hi]}
        )
    res = run_bass_kernel_spmd(nc, in_maps, core_ids=list(range(N_CORES)))
    return np.concatenate(
        [res.results[c]["out"] for c in range(N_CORES)], axis=0
    ).astype(np.float32)


# revision 21
# speedup vs baseline: 1.5295x; 1.5295x over previous
"""DepthScaleShiftInvLoss kernel for one TRN2 chip (8 NeuronCores).

Full inputs: pred/gt f32 [32,512,512], mask bool [32,512,512].
Output: dense masked loss f32 [32,512,512] (zeros off-mask).

Sharding: pure data parallel — batch dim split 4 samples/core across 8 cores,
no cross-core communication.

History: baseline 76us (f32 I/O, all DMA on one queue, all stats passes on
ACT) -> v2 25.7us (bf16 I/O halves traffic; DMA split over both HWDGE
queues; fused stage-C bias) -> v4 (this file): the v2 profile was
ACT-engine-bound (4 big activation passes/sample), so the abs-sum passes
move to DVE as scalar_tensor_tensor((x - shift) * mf) + tensor_reduce(abs)
— which also deletes the off-mask correction terms — stage-C's two
elementwise ops fuse into one scalar_tensor_tensor, and the final mask
multiply runs on the otherwise-idle GpSimd (Pool) engine. ACT keeps two
big passes per sample (mask u8->bf16 cast with count accumulation, and the
fused |a*x + b| output pass).

Per-core pipeline (SBUF layout per sample is [128 partitions x 2048],
partition p holding image rows [4p, 4p+4); samples are independent chains,
emitted staggered so one sample's stats barriers overlap other samples'
bulk work):

  stage A   ACT: mf = bf16(mask_u8) (Copy, accum -> count)
            DVE: pm = pred*mf, gm = gt*mf (bf16)
            PE:  masked sums of pm/gm via 16 accumulating matmuls each
                 (data stationary, ones moving) -> PSUM [128,1] partials
  barrier1  PE: ones[128,128] @ partials -> totals replicated across
            partitions; cnt=max(c,1), invc=1/cnt, sp, sg (tiny DVE ops).
  stage B   DVE: cen = (pm - sp)*mf   (scalar_tensor_tensor; off-mask 0)
            DVE: tensor_reduce abs-sum -> per-partition partials
  barrier2  PE matmul folds partials; scp=max(sum*invc,EPS), a=1/scp,
            r=scp/scg, bias2=a*(sp-r*sg) (tiny DVE ops).
  stage C   DVE: w = r*gm - pm        (scalar_tensor_tensor)
            ACT: o1 = |a*w + bias2|   (== a*|pm - sp - r*(gt - sg)| on-mask)
            Pool: out = o1*mf -> bf16, DMA out
"""

import numpy as np
import ml_dtypes

import concourse.bass as bass
import concourse.bacc as bacc
import concourse.tile as tile
from concourse import mybir
from concourse.bass_utils import run_bass_kernel_spmd

B, H, W = 32, 512, 512
N_CORES = 8
B_LOC = B // N_CORES          # samples per core
P = 128                       # SBUF partitions
FD = (H // P) * W             # free elements per sample per partition
N_ELEM = float(H * W)         # elements per sample
EPS = 1e-6

f32 = mybir.dt.float32
bf16 = mybir.dt.bfloat16
u8 = mybir.dt.uint8

ALU = mybir.AluOpType
ACTF = mybir.ActivationFunctionType
AXL = mybir.AxisListType

# engine placement knobs (tuned empirically). Bring-up lesson: the
# scalar_tensor_tensor / tensor_reduce family on DVE is far slower on real
# silicon than the cost model thinks (v4 with 5 of them: 99-121us vs 25.7us)
# — stick to plain tensor_tensor/tensor_scalar for bulk DVE work.
ABS_P = "act"        # "dve": STT+reduce   "act": activation Abs+accum
ABS_G = "act"
OUTF = "dve"         # final mask multiply on "pool" or "dve"
MASK_VIA = "dma"     # "dma": gpsimd casting DMA + PE count; "act": Copy+accum


class _PerSample:
    __slots__ = ("mf", "pm", "gm", "prt", "psum_s", "p2",
                 "cnt", "invc", "sp", "sg", "a_p", "r_t", "bias2",
                 "corr_p", "corr_g")


def build_body(nc):
    pred = nc.dram_tensor("pred", [B_LOC, H, W], bf16, kind="ExternalInput").ap()
    gt = nc.dram_tensor("gt", [B_LOC, H, W], bf16, kind="ExternalInput").ap()
    mask = nc.dram_tensor("mask", [B_LOC, H, W], u8, kind="ExternalInput").ap()
    out = nc.dram_tensor("out", [B_LOC, H, W], bf16, kind="ExternalOutput").ap()

    # [a, (p r), w] -> [p, a, (r w)]: per (partition, sample) 2048 contiguous
    # elements in DRAM.
    pr = pred.rearrange("a (p r) w -> p a (r w)", p=P)
    gr = gt.rearrange("a (p r) w -> p a (r w)", p=P)
    mr = mask.rearrange("a (p r) w -> p a (r w)", p=P)
    outr = out.rearrange("a (p r) w -> p a (r w)", p=P)

    LAST = B_LOC - 1

    with tile.TileContext(nc) as tc:
        with (
            tc.tile_pool(name="io", bufs=3) as io,
            tc.tile_pool(name="keep", bufs=B_LOC) as keep,
            tc.tile_pool(name="tmp", bufs=2) as tmp,
            tc.tile_pool(name="small", bufs=B_LOC) as small,
            tc.tile_pool(name="ps", bufs=2, space="PSUM") as ps,
            tc.tile_pool(name="const", bufs=1) as const,
        ):
            ones = const.tile([P, P], f32)
            nc.vector.memset(ones, 1.0)
            ones_b = const.tile([P, 1], bf16)
            nc.vector.memset(ones_b, 1.0)

            def pe_sum(big, psum_acc):
                # total-sum helper: 16 accumulating matmuls with the data as
                # the stationary operand; psum_acc[m] = sum_p,k big[p, 128k+m]
                for k in range(0, FD, P):
                    nc.tensor.matmul(psum_acc, big[:, k:k + P], ones_b,
                                     start=(k == 0), stop=(k == FD - P))

            S = [_PerSample() for _ in range(B_LOC)]

            def stage_mask(s):
                st = S[s]
                st.mf = keep.tile([P, FD], bf16, tag="mf", name=f"mf{s}")
                st.psum_s = ps.tile([P, 3], f32, tag="psum_s", name=f"pss{s}")
                if MASK_VIA == "dma":
                    # SWDGE casting DMA: u8 DRAM -> bf16 SBUF, no compute
                    # pass; count via PE masked-sum instead
                    nc.gpsimd.dma_start(out=st.mf, in_=mr[:, s, :])
                    pe_sum(st.mf, st.psum_s[:, 0:1])
                else:
                    m_in = io.tile([P, FD], u8, tag="m_in", bufs=B_LOC,
                                   name=f"m_in{s}")
                    nc.sync.dma_start(out=m_in, in_=mr[:, s, :])
                    st.prt = small.tile([P, 3], f32, tag="prt",
                                        name=f"prt{s}")
                    nc.scalar.activation(out=st.mf, in_=m_in, func=ACTF.Copy,
                                         accum_out=st.prt[:, 0:1])

            def stage_a(s):
                st = S[s]
                p_in = io.tile([P, FD], bf16, tag="p_in", name=f"p_in{s}")
                nc.sync.dma_start(out=p_in, in_=pr[:, s, :])
                g_in = io.tile([P, FD], bf16, tag="g_in", name=f"g_in{s}")
                nc.scalar.dma_start(out=g_in, in_=gr[:, s, :])

                st.pm = keep.tile([P, FD], bf16, tag="pm", name=f"pm{s}")
                nc.vector.tensor_tensor(st.pm, p_in, st.mf, ALU.mult)
                st.gm = keep.tile([P, FD], bf16, tag="gm", name=f"gm{s}")
                nc.vector.tensor_tensor(st.gm, g_in, st.mf, ALU.mult)
                # masked sums on the TensorEngine (per-partition partials)
                pe_sum(st.pm, st.psum_s[:, 1:2])
                pe_sum(st.gm, st.psum_s[:, 2:3])
                if MASK_VIA == "dma":
                    st.prt = small.tile([P, 3], f32, tag="prt",
                                        name=f"prt{s}")
                    nc.scalar.copy(out=st.prt, in_=st.psum_s)
                else:
                    nc.scalar.copy(out=st.prt[:, 1:3], in_=st.psum_s[:, 1:3])

            def barrier1(s):
                st = S[s]
                late = s == LAST
                psum1 = ps.tile([P, 3], f32, tag="psum1", name=f"ps1_{s}")
                nc.tensor.matmul(psum1, ones, st.prt, start=True, stop=True)
                stats1 = small.tile([P, 3], f32, tag="stats1", name=f"st1_{s}")
                if late:
                    nc.vector.tensor_copy(stats1, psum1)
                else:
                    nc.scalar.copy(out=stats1, in_=psum1)

                st.cnt = small.tile([P, 1], f32, tag="cnt", name=f"cnt{s}")
                st.invc = small.tile([P, 1], f32, tag="invc", name=f"invc{s}")
                st.sp = small.tile([P, 1], f32, tag="sp", name=f"sp{s}")
                st.sg = small.tile([P, 1], f32, tag="sg", name=f"sg{s}")
                eng = nc.vector
                eng.tensor_scalar(st.cnt, stats1[:, 0:1], 1.0, None, ALU.max)
                nc.vector.reciprocal(st.invc, st.cnt)
                eng.tensor_tensor(st.sp, stats1[:, 1:2], st.invc, ALU.mult)
                eng.tensor_tensor(st.sg, stats1[:, 2:3], st.invc, ALU.mult)

            def _abs_dve(st, s, src, shift, dst):
                cen = tmp.tile([P, FD], bf16, tag="cen", name=f"cen{s}_{id(src)%97}")
                nc.vector.scalar_tensor_tensor(cen, src, shift, st.mf,
                                               ALU.subtract, ALU.mult)
                nc.vector.tensor_reduce(dst, cen, AXL.XYZW, ALU.add,
                                        apply_absolute_value=True)

            def _abs_act(st, s, src, shift, dst, corr_attr):
                # activation path needs the off-mask correction: accum
                # counts (N-cnt) copies of |shift|
                scr = tmp.tile([P, FD], bf16, tag="scr", name=f"scr{s}_{id(src)%97}")
                nc.scalar.activation(out=scr, in_=src, func=ACTF.Abs,
                                     bias=shift, scale=-1.0, accum_out=dst)
                ash = small.tile([P, 1], f32, tag=f"ash{corr_attr}",
                                 name=f"ash{corr_attr}{s}")
                nc.scalar.activation(out=ash, in_=shift, func=ACTF.Abs)
                offc = small.tile([P, 1], f32, tag=f"offc{corr_attr}",
                                  name=f"offc{corr_attr}{s}")
                nc.vector.tensor_scalar(offc, st.cnt, -1.0, N_ELEM,
                                        ALU.mult, ALU.add)
                corr = small.tile([P, 1], f32, tag=f"corr{corr_attr}",
                                  name=f"corr{corr_attr}{s}")
                nc.vector.tensor_tensor(corr, offc, ash, ALU.mult)
                setattr(st, corr_attr, corr)

            def stage_b(s):
                st = S[s]
                st.p2 = small.tile([P, 2], f32, tag="p2", name=f"p2_{s}")
                if ABS_P == "dve":
                    _abs_dve(st, s, st.pm, st.sp, st.p2[:, 0:1])
                    st.corr_p = None
                else:
                    _abs_act(st, s, st.pm, st.sp, st.p2[:, 0:1], "corr_p")
                if ABS_G == "dve":
                    _abs_dve(st, s, st.gm, st.sg, st.p2[:, 1:2])
                    st.corr_g = None
                else:
                    _abs_act(st, s, st.gm, st.sg, st.p2[:, 1:2], "corr_g")

            def barrier2(s):
                st = S[s]
                late = s == LAST
                psum2 = ps.tile([P, 2], f32, tag="psum2", name=f"ps2_{s}")
                nc.tensor.matmul(psum2, ones, st.p2, start=True, stop=True)
                stats2 = small.tile([P, 2], f32, tag="stats2", name=f"st2_{s}")
                if late:
                    nc.vector.tensor_copy(stats2, psum2)
                else:
                    nc.scalar.copy(out=stats2, in_=psum2)

                eng = nc.vector
                sums = [stats2[:, 0:1], stats2[:, 1:2]]
                for i, corr in enumerate([st.corr_p, st.corr_g]):
                    if corr is not None:
                        num = small.tile([P, 1], f32, tag=f"num{i}",
                                         name=f"num{i}_{s}")
                        eng.tensor_tensor(num, sums[i], corr, ALU.subtract)
                        sums[i] = num
                scp = small.tile([P, 1], f32, tag="scp", name=f"scp{s}")
                eng.tensor_scalar(scp, sums[0], st.invc, EPS, ALU.mult, ALU.max)
                scg = small.tile([P, 1], f32, tag="scg", name=f"scg{s}")
                eng.tensor_scalar(scg, sums[1], st.invc, EPS, ALU.mult, ALU.max)
                st.a_p = small.tile([P, 1], f32, tag="a_p", name=f"ap{s}")
                nc.vector.reciprocal(st.a_p, scp)
                i_g = small.tile([P, 1], f32, tag="i_g", name=f"ig{s}")
                nc.vector.reciprocal(i_g, scg)
                st.r_t = small.tile([P, 1], f32, tag="r_t", name=f"rt{s}")
                eng.tensor_tensor(st.r_t, scp, i_g, ALU.mult)
                rsg = small.tile([P, 1], f32, tag="rsg", name=f"rsg{s}")
                eng.tensor_tensor(rsg, st.r_t, st.sg, ALU.mult)
                # bias2 = a*(r*sg - sp): o1 = |a*(pm - r*gm) + bias2|
                qd = small.tile([P, 1], f32, tag="qd", name=f"qd{s}")
                eng.tensor_tensor(qd, rsg, st.sp, ALU.subtract)
                st.bias2 = small.tile([P, 1], f32, tag="bias2", name=f"b2{s}")
                eng.tensor_tensor(st.bias2, qd, st.a_p, ALU.mult)

            def stage_c(s):
                st = S[s]
                u = tmp.tile([P, FD], bf16, tag="u", name=f"u{s}")
                nc.vector.tensor_scalar(u, st.gm, st.r_t, None, ALU.mult)
                w = tmp.tile([P, FD], bf16, tag="w", name=f"w{s}")
                nc.vector.tensor_tensor(w, st.pm, u, ALU.subtract)
                o1 = tmp.tile([P, FD], bf16, tag="o1", name=f"o1{s}")
                nc.scalar.activation(out=o1, in_=w, func=ACTF.Abs,
                                     scale=st.a_p, bias=st.bias2)
                outf = tmp.tile([P, FD], bf16, tag="outf", name=f"outf{s}")
                if OUTF == "pool":
                    nc.gpsimd.tensor_tensor(outf, o1, st.mf, ALU.mult)
                else:
                    nc.vector.tensor_tensor(outf, o1, st.mf, ALU.mult)
                if s % 2 == 0:
                    nc.sync.dma_start(out=outr[:, s, :], in_=outf)
                else:
                    nc.scalar.dma_start(out=outr[:, s, :], in_=outf)

            # Emission order == scheduling priority. Masks/casts first, the
            # pred/gt stream + TTRs next, then each sample's stats/B staged
            # ahead of earlier samples' C.
            for s in range(B_LOC):
                stage_mask(s)
            stage_a(0)
            stage_a(1)
            barrier1(0)
            stage_b(0)
            stage_a(2)
            barrier1(1)
            stage_b(1)
            barrier2(0)
            stage_a(3)
            barrier1(2)
            stage_b(2)
            barrier2(1)
            stage_c(0)
            barrier1(3)
            stage_b(3)
            barrier2(2)
            stage_c(1)
            barrier2(3)
            stage_c(2)
            stage_c(3)
    return nc


_CACHED = None


def _get_nc():
    global _CACHED
    if _CACHED is None:
        nc = bacc.Bacc("TRN2", target_bir_lowering=False, debug=False)
        build_body(nc)
        nc.compile()
        _CACHED = nc
    return _CACHED


def kernel(pred: np.ndarray, gt: np.ndarray, mask: np.ndarray) -> np.ndarray:
    pred = np.ascontiguousarray(np.asarray(pred).astype(ml_dtypes.bfloat16))
    gt = np.ascontiguousarray(np.asarray(gt).astype(ml_dtypes.bfloat16))
    mask = np.asarray(mask)
    mask_u8 = np.ascontiguousarray(
        mask.view(np.uint8) if mask.dtype == np.bool_ else mask.astype(np.uint8)
    )

    nc = _get_nc()
    in_maps = []
    for c in range(N_CORES):
        lo, hi = c * B_LOC, (c + 1) * B_LOC
        in_maps.append(
            {"pred": pred[lo:hi], "gt": gt[lo:hi], "mask": mask_u8[lo:hi]}
        )
    res = run_bass_kernel_spmd(nc, in_maps, core_ids=list(range(N_CORES)))
    return np.concatenate(
        [res.results[c]["out"] for c in range(N_CORES)], axis=0
    ).astype(np.float32)
